# revision 31
# baseline (speedup 1.0000x reference)
"""DTCWT inverse (qshift, single level) as a Bass/Tile kernel for TRN2.

Formulation per channel (128x128 -> 256x256):
    out = C0 @ Yl @ C0^T + C1 @ lh @ C0^T + C0 @ hl @ C1^T + C1 @ hh @ C1^T
where lh/hl/hh are the c2q quad images (built on HOST from the 6 complex
subbands - linear, so it folds into input packing) and C0/C1 are the 256x128
banded colifilt synthesis matrices.

Device pipeline per channel:
  stage 1 (height): psum b[128 cols, 512] = [C0@Yl+C1@lh | C0@hl+C1@hh]^T
      4 matmuls, data stationary [128,128], statics moving [128,256].
  copy1: b -> tts fp16 (single aligned [128,512] cast, ACT/DVE alternating)
  stage 2 (width): psum yb[128 j, 512] from tts slices stationary, same
      statics moving. 4 matmuls.
  copy2: yb -> YB fp16 (other engine), group-level DMA out.

Startup: channel-major input packing lets the first DMA carry only ch0
(128KB) so the PE starts ~3us earlier; a few warmup matmuls on scratch
data pre-ramp the PE clock (p-state) during the load window. PE order is
software-pipelined: stage2(c) emitted after stage1(c+2) so the copy1
chain never stalls the tensor engine. All device data fp16.
"""
import numpy as np

import concourse.bacc as bacc
import concourse.tile as tile
from concourse import mybir

F16 = mybir.dt.float16
F32 = mybir.dt.float32

# ---------------- host-side static matrix construction ----------------

_H0A = np.array([0.0351638365171441, 0.0, -0.0883294244510729,
                 0.233890320607236, 0.760272369066126, 0.587518297723561,
                 0.0, -0.114301837144249, 0.0, 0.0], dtype=np.float64)
_H0B = _H0A[::-1].copy()
_ALT = (-1.0) ** np.arange(10)
_H1A = _H0B * _ALT
_H1B = _H1A[::-1].copy()
G0A, G0B, G1A, G1B = _H0B, _H0A, _H1B, _H1A


def _reflect(x, minx, maxx):
    x = np.asarray(x, dtype=np.float64)
    rng = maxx - minx
    rng2 = 2.0 * rng
    mod = np.fmod(x - minx, rng2)
    normed = np.where(mod < 0, mod + rng2, mod)
    return (np.where(normed >= rng, rng2 - normed, normed) + minx).astype(np.int64)


def _colifilt_matrix(ha, hb, r=128):
    """C (2r x r) with colifilt(X) = C @ X."""
    m = ha.shape[0]
    m2 = m // 2
    xe = _reflect(np.arange(-m2, r + m2), -0.5, r - 0.5)
    t = np.arange(2, r + m - 1, 2)
    if float(np.sum(ha * hb)) > 0:
        ta, tb = t, t - 1
    else:
        ta, tb = t - 1, t
    r2 = r // 2
    hao, hae = ha[0::2], ha[1::2]
    hbo, hbe = hb[0::2], hb[1::2]

    def vconv_mat(sel_idx, h):
        hf = h[::-1]
        M = np.zeros((r2, r), dtype=np.float64)
        for i in range(r2):
            for k in range(m2):
                M[i, sel_idx[i + k]] += hf[k]
        return M

    C = np.zeros((2 * r, r), dtype=np.float64)
    C[0::4] = vconv_mat(xe[tb], hao)
    C[1::4] = vconv_mat(xe[ta], hbo)
    C[2::4] = vconv_mat(xe[tb], hae)
    C[3::4] = vconv_mat(xe[ta], hbe)
    return C


def build_statics():
    """ST (128 x 512) fp16 = [C0^T | C1^T]; second element kept for
    interface compatibility (unused)."""
    C0 = _colifilt_matrix(G0B, G0A)
    C1 = _colifilt_matrix(G1B, G1A)
    ST = np.concatenate([C0.T, C1.T], axis=1).astype(np.float16)
    return np.ascontiguousarray(ST), None


# ---------------- host-side input/output packing ----------------

def _c2q(wr, wi):
    """bands (C, 2, 64, 64) -> quad (C, 128, 128), scaled by 1/sqrt(2)."""
    w1r, w2r = wr[:, 0], wr[:, 1]
    w1i, w2i = wi[:, 0], wi[:, 1]
    x1 = w1r + w2r
    x2 = w1i + w2i
    x3 = w1i - w2i
    x4 = w2r - w1r
    c, h, w = x1.shape
    top = np.stack([x1, x2], axis=-1).reshape(c, h, 2 * w)
    bot = np.stack([x3, x4], axis=-1).reshape(c, h, 2 * w)
    y = np.stack([top, bot], axis=-2).reshape(c, 2 * h, 2 * w)
    return y * np.float32(1.0 / np.sqrt(2.0))


def pack_in(Yl, Yhr, Yhi, ST):
    """-> YALL [128, 33280] fp16 = [ST (512) | 64 channels x 512], channel c
    block = [Yl_c | lh_c | hl_c | hh_c] (128 cols each), channel-major."""
    lh = _c2q(Yhr[:, 0:6:5], Yhi[:, 0:6:5])
    hl = _c2q(Yhr[:, 2:4:1], Yhi[:, 2:4:1])
    hh = _c2q(Yhr[:, 1:5:3], Yhi[:, 1:5:3])
    qs = np.stack([lh, hl, hh], axis=1)         # [64 c, 3 q, 128 p, 128 w]
    yl = Yl.transpose(1, 0, 2).reshape(128, 64, 1, 128)
    A = np.concatenate([yl, qs.transpose(2, 0, 1, 3)], axis=2)  # [p, c, 4, w]
    A = A.reshape(128, 64 * 512).astype(np.float16)
    return np.ascontiguousarray(np.concatenate([ST, A], axis=1))


def unpack_y(Y_RAW):
    """[128, 8, 8, 512] fp16 raw -> [64, 256, 256] f32.
    Y[c, h*128+p, n] = Y_RAW[p, c//8, c%8, h*256+n]."""
    Y = Y_RAW.reshape(128, 64, 2, 256).transpose(1, 2, 0, 3)
    return np.ascontiguousarray(Y.reshape(64, 256, 256).astype(np.float32))


# ---------------- device kernel ----------------

def build_kernel(n_ch=64, G=8, n_cores=8, lookahead=2, warmup=14):
    nc = bacc.Bacc("TRN2", target_bir_lowering=False, debug=False,
                   num_devices=n_cores)
    YALL = nc.dram_tensor("YALL", [128, 33280], F16,
                          kind="ExternalInput").ap()
    OUT = nc.dram_tensor("Y", [128, 8, 4096], F16, kind="ExternalOutput").ap()

    n_groups = n_ch // G          # 8
    with tile.TileContext(nc) as tc:
        with (
            tc.tile_pool(name="const", bufs=1) as const,
            tc.tile_pool(name="inp", bufs=3) as inp,
            tc.tile_pool(name="tt", bufs=6) as ttp,
            tc.tile_pool(name="yout", bufs=3) as yp,
            tc.tile_pool(name="psb", bufs=4, space="PSUM") as pb,
            tc.tile_pool(name="psy", bufs=4, space="PSUM") as py,
        ):
            # D0: statics + ch0 in one transfer so the PE can start early
            g0a = const.tile([128, 1024], F16)
            nc.sync.dma_start(g0a[:], YALL[:, 0:1024])
            S0 = g0a[:, 0:256]
            S1 = g0a[:, 256:512]
            # D1: ch1-2, D2: ch3-4, D3: ch5-7 (staged to stay ahead of PE)
            g0b = inp.tile([128, 1024], F16, tag="g0b")
            nc.sync.dma_start(g0b[:], YALL[:, 1024:2048])
            g0c = inp.tile([128, 1024], F16, tag="g0c")
            nc.sync.dma_start(g0c[:], YALL[:, 2048:3072])
            g0d = inp.tile([128, 1536], F16, tag="g0d")
            nc.sync.dma_start(g0d[:], YALL[:, 3072:4608])

            # scratch for PE warmup (clock ramp while inputs load)
            wsrc = const.tile([128, 256], F16)
            nc.gpsimd.memset(wsrc[:], 0)

            groups = [None, None, None]

            def load_group(g, split=False):
                t = inp.tile([128, 4096], F16, tag="gin")
                base = 512 + g * 4096
                if split:
                    # two halves: first channels land sooner in the
                    # DMA-saturated startup window
                    nc.sync.dma_start(t[:, 0:2048], YALL[:, base:base + 2048])
                    nc.sync.dma_start(t[:, 2048:4096],
                                      YALL[:, base + 2048:base + 4096])
                else:
                    nc.sync.dma_start(t[:], YALL[:, base:base + 4096])
                groups[g % 3] = t



            def ch_slices(k):
                g, ci = divmod(k, G)
                if g == 0:
                    if ci == 0:
                        base, tl = 512, g0a
                    elif ci < 3:
                        base, tl = (ci - 1) * 512, g0b
                    elif ci < 5:
                        base, tl = (ci - 3) * 512, g0c
                    else:
                        base, tl = (ci - 5) * 512, g0d
                else:
                    base, tl = ci * 512, groups[g % 3]
                return [tl[:, base + q * 128: base + (q + 1) * 128]
                        for q in range(4)]

            YBs = {}
            state = {}

            def stage1(k):
                yl_s, lh_s, hl_s, hh_s = ch_slices(k)
                b = pb.tile([128, 512], F32, tag="b")
                nc.tensor.matmul(b[:, 0:256], yl_s, S0,
                                 start=True, stop=False, skip_group_check=True)
                nc.tensor.matmul(b[:, 0:256], lh_s, S1,
                                 start=False, stop=True, skip_group_check=True)
                nc.tensor.matmul(b[:, 256:512], hl_s, S0,
                                 start=True, stop=False, skip_group_check=True)
                nc.tensor.matmul(b[:, 256:512], hh_s, S1,
                                 start=False, stop=True, skip_group_check=True)
                t = ttp.tile([128, 512], F16, tag="t")
                if k % 2 == 0:
                    nc.scalar.copy(t[:], b[:])
                else:
                    nc.vector.tensor_copy(t[:], b[:])
                state[k] = t

            def stage2(k):
                g, ci = divmod(k, G)
                t = state.pop(k)
                yb = py.tile([128, 512], F32, tag="yb")
                nc.tensor.matmul(yb[:, 0:256], t[:, 0:128], S0,
                                 start=True, stop=False, skip_group_check=True)
                nc.tensor.matmul(yb[:, 0:256], t[:, 256:384], S1,
                                 start=False, stop=True, skip_group_check=True)
                nc.tensor.matmul(yb[:, 256:512], t[:, 128:256], S0,
                                 start=True, stop=False, skip_group_check=True)
                nc.tensor.matmul(yb[:, 256:512], t[:, 384:512], S1,
                                 start=False, stop=True, skip_group_check=True)
                if ci == 0:
                    YBs[g] = yp.tile([128, 4096], F16, name=f"ybo{g}",
                                     tag="yb_out")
                YB = YBs[g]
                o = ci * 512
                if k >= n_ch - 2:
                    # drain: halve the last copies across both engines
                    nc.scalar.copy(YB[:, o:o + 256], yb[:, 0:256])
                    nc.vector.tensor_copy(YB[:, o + 256:o + 512], yb[:, 256:512])
                elif k % 2 == 0:
                    nc.vector.tensor_copy(YB[:, o:o + 512], yb[:])
                else:
                    nc.scalar.copy(YB[:, o:o + 512], yb[:])
                last = g == n_groups - 1
                if ci == G // 2 - 1:
                    nc.sync.dma_start(OUT[:, g, 0:2048], YB[:, 0:2048])
                elif last and ci == G - 3:
                    nc.sync.dma_start(OUT[:, g, 2048:3072], YB[:, 2048:3072])
                elif last and ci == G - 2:
                    nc.sync.dma_start(OUT[:, g, 3072:3584], YB[:, 3072:3584])
                elif ci == G - 1:
                    if last:
                        # final store issued from ACT (hw-DGE capable): runs
                        # in parallel with Sync's previous issue, skips a
                        # sem hop at the drain
                        nc.scalar.dma_start(OUT[:, g, 3584:4096],
                                            YB[:, 3584:4096])
                    else:
                        nc.sync.dma_start(OUT[:, g, 2048:4096],
                                          YB[:, 2048:4096])
                    YBs.pop(g)

            # PE warmup: ramp the clock while ch0/statics stream in
            for w in range(warmup):
                wb = pb.tile([128, 512], F32, tag="b")
                nc.tensor.matmul(wb[:, 0:256], wsrc[:, 0:128], wsrc[:, 0:256],
                                 start=True, stop=True, skip_group_check=True)

            total = n_ch
            for k in range(total + lookahead):
                if k < total:
                    g, ci = divmod(k, G)
                    if ci == 0 and g + 1 < n_groups:
                        load_group(g + 1, split=(g == 0))
                    stage1(k)
                j = k - lookahead
                if j >= 0:
                    stage2(j)

    nc.compile()
    return nc


# ---------------- host wrapper: shard, run on 8 cores, gather ----------------

_CACHED = {}


def _get_compiled():
    if "nc" not in _CACHED:
        _CACHED["nc"] = build_kernel()
        _CACHED["stats"] = build_statics()
    return _CACHED["nc"], _CACHED["stats"]


def make_in_map(Yl_b, Yhr_b, Yhi_b, ST, SIGNS=None):
    return {"YALL": pack_in(Yl_b, Yhr_b, Yhi_b, ST)}


def kernel(Yl, Yhr, Yhi):
    """Inverse DTCWT (qshift) level. Yl (8,64,128,128) f32,
    Yhr/Yhi (8,64,6,64,64) f32 -> (8,64,256,256) f32.
    Data-parallel over batch: one batch element per NeuronCore."""
    from concourse.bass_utils import run_bass_kernel_spmd

    Yl = np.asarray(Yl, dtype=np.float32)
    Yhr = np.asarray(Yhr, dtype=np.float32)
    Yhi = np.asarray(Yhi, dtype=np.float32)
    B = Yl.shape[0]
    assert B == 8, f"expected batch 8, got {B}"

    nc, (ST, SIGNS) = _get_compiled()
    in_maps = [make_in_map(Yl[b], Yhr[b], Yhi[b], ST, SIGNS)
               for b in range(B)]
    res = run_bass_kernel_spmd(nc, in_maps, core_ids=list(range(B)))
    out = np.stack([unpack_y(res.results[b]["Y"].reshape(128, 8, 8, 512))
                    for b in range(B)])
    return out


# revision 32
# speedup vs baseline: 1.0150x; 1.0150x over previous
"""DTCWT inverse (qshift, single level) as a Bass/Tile kernel for TRN2.

Formulation per channel (128x128 -> 256x256):
    out = C0 @ Yl @ C0^T + C1 @ lh @ C0^T + C0 @ hl @ C1^T + C1 @ hh @ C1^T
where lh/hl/hh are the c2q quad images (built on HOST from the 6 complex
subbands - linear, so it folds into input packing) and C0/C1 are the 256x128
banded colifilt synthesis matrices.

Device pipeline per channel:
  stage 1 (height): psum b[128 cols, 512] = [C0@Yl+C1@lh | C0@hl+C1@hh]^T
      4 matmuls, data stationary [128,128], statics moving [128,256].
  copy1: b -> tts fp16 (single aligned [128,512] cast, ACT/DVE alternating)
  stage 2 (width): psum yb[128 j, 512] from tts slices stationary, same
      statics moving. 4 matmuls.
  copy2: yb -> YB fp16 (other engine), group-level DMA out.

Startup: channel-major input packing lets the first DMA carry only ch0
(128KB) so the PE starts ~3us earlier; a few warmup matmuls on scratch
data pre-ramp the PE clock (p-state) during the load window. PE order is
software-pipelined: stage2(c) emitted after stage1(c+2) so the copy1
chain never stalls the tensor engine. All device data fp16.
"""
import numpy as np

import concourse.bacc as bacc
import concourse.tile as tile
from concourse import mybir

F16 = mybir.dt.float16
F32 = mybir.dt.float32

# ---------------- host-side static matrix construction ----------------

_H0A = np.array([0.0351638365171441, 0.0, -0.0883294244510729,
                 0.233890320607236, 0.760272369066126, 0.587518297723561,
                 0.0, -0.114301837144249, 0.0, 0.0], dtype=np.float64)
_H0B = _H0A[::-1].copy()
_ALT = (-1.0) ** np.arange(10)
_H1A = _H0B * _ALT
_H1B = _H1A[::-1].copy()
G0A, G0B, G1A, G1B = _H0B, _H0A, _H1B, _H1A


def _reflect(x, minx, maxx):
    x = np.asarray(x, dtype=np.float64)
    rng = maxx - minx
    rng2 = 2.0 * rng
    mod = np.fmod(x - minx, rng2)
    normed = np.where(mod < 0, mod + rng2, mod)
    return (np.where(normed >= rng, rng2 - normed, normed) + minx).astype(np.int64)


def _colifilt_matrix(ha, hb, r=128):
    """C (2r x r) with colifilt(X) = C @ X."""
    m = ha.shape[0]
    m2 = m // 2
    xe = _reflect(np.arange(-m2, r + m2), -0.5, r - 0.5)
    t = np.arange(2, r + m - 1, 2)
    if float(np.sum(ha * hb)) > 0:
        ta, tb = t, t - 1
    else:
        ta, tb = t - 1, t
    r2 = r // 2
    hao, hae = ha[0::2], ha[1::2]
    hbo, hbe = hb[0::2], hb[1::2]

    def vconv_mat(sel_idx, h):
        hf = h[::-1]
        M = np.zeros((r2, r), dtype=np.float64)
        for i in range(r2):
            for k in range(m2):
                M[i, sel_idx[i + k]] += hf[k]
        return M

    C = np.zeros((2 * r, r), dtype=np.float64)
    C[0::4] = vconv_mat(xe[tb], hao)
    C[1::4] = vconv_mat(xe[ta], hbo)
    C[2::4] = vconv_mat(xe[tb], hae)
    C[3::4] = vconv_mat(xe[ta], hbe)
    return C


def build_statics():
    """ST (128 x 512) fp16 = [C0^T | C1^T]; second element kept for
    interface compatibility (unused)."""
    C0 = _colifilt_matrix(G0B, G0A)
    C1 = _colifilt_matrix(G1B, G1A)
    ST = np.concatenate([C0.T, C1.T], axis=1).astype(np.float16)
    return np.ascontiguousarray(ST), None


# ---------------- host-side input/output packing ----------------

def _c2q(wr, wi):
    """bands (C, 2, 64, 64) -> quad (C, 128, 128), scaled by 1/sqrt(2)."""
    w1r, w2r = wr[:, 0], wr[:, 1]
    w1i, w2i = wi[:, 0], wi[:, 1]
    x1 = w1r + w2r
    x2 = w1i + w2i
    x3 = w1i - w2i
    x4 = w2r - w1r
    c, h, w = x1.shape
    top = np.stack([x1, x2], axis=-1).reshape(c, h, 2 * w)
    bot = np.stack([x3, x4], axis=-1).reshape(c, h, 2 * w)
    y = np.stack([top, bot], axis=-2).reshape(c, 2 * h, 2 * w)
    return y * np.float32(1.0 / np.sqrt(2.0))


def pack_in(Yl, Yhr, Yhi, ST):
    """-> YALL [128, 33280] fp16 = [ST (512) | 64 channels x 512], channel c
    block = [Yl_c | lh_c | hl_c | hh_c] (128 cols each), channel-major."""
    lh = _c2q(Yhr[:, 0:6:5], Yhi[:, 0:6:5])
    hl = _c2q(Yhr[:, 2:4:1], Yhi[:, 2:4:1])
    hh = _c2q(Yhr[:, 1:5:3], Yhi[:, 1:5:3])
    qs = np.stack([lh, hl, hh], axis=1)         # [64 c, 3 q, 128 p, 128 w]
    yl = Yl.transpose(1, 0, 2).reshape(128, 64, 1, 128)
    A = np.concatenate([yl, qs.transpose(2, 0, 1, 3)], axis=2)  # [p, c, 4, w]
    A = A.reshape(128, 64 * 512).astype(np.float16)
    return np.ascontiguousarray(np.concatenate([ST, A], axis=1))


def unpack_y(Y_RAW):
    """[128, 8, 8, 512] fp16 raw -> [64, 256, 256] f32.
    Y[c, h*128+p, n] = Y_RAW[p, c//8, c%8, h*256+n]."""
    Y = Y_RAW.reshape(128, 64, 2, 256).transpose(1, 2, 0, 3)
    return np.ascontiguousarray(Y.reshape(64, 256, 256).astype(np.float32))


# ---------------- device kernel ----------------

def build_kernel(n_ch=64, G=8, n_cores=8, lookahead=2, warmup=14):
    nc = bacc.Bacc("TRN2", target_bir_lowering=False, debug=False,
                   num_devices=n_cores)
    YALL = nc.dram_tensor("YALL", [128, 33280], F16,
                          kind="ExternalInput").ap()
    OUT = nc.dram_tensor("Y", [128, 8, 4096], F16, kind="ExternalOutput").ap()

    n_groups = n_ch // G          # 8
    with tile.TileContext(nc) as tc:
        with (
            tc.tile_pool(name="const", bufs=1) as const,
            tc.tile_pool(name="inp", bufs=3) as inp,
            tc.tile_pool(name="tt", bufs=6) as ttp,
            tc.tile_pool(name="yout", bufs=3) as yp,
            tc.tile_pool(name="psb", bufs=4, space="PSUM") as pb,
            tc.tile_pool(name="psy", bufs=4, space="PSUM") as py,
        ):
            # D0: statics + ch0 in one transfer so the PE can start early
            g0a = const.tile([128, 1024], F16)
            nc.sync.dma_start(g0a[:], YALL[:, 0:1024])
            S0 = g0a[:, 0:256]
            S1 = g0a[:, 256:512]
            # D1: ch1-2, D2: ch3-4, D3: ch5-7 (staged to stay ahead of PE)
            g0b = inp.tile([128, 1024], F16, tag="g0b")
            nc.sync.dma_start(g0b[:], YALL[:, 1024:2048])
            g0c = inp.tile([128, 1024], F16, tag="g0c")
            nc.sync.dma_start(g0c[:], YALL[:, 2048:3072])
            g0d = inp.tile([128, 1536], F16, tag="g0d")
            nc.sync.dma_start(g0d[:], YALL[:, 3072:4608])

            # scratch for PE warmup (clock ramp while inputs load)
            wsrc = const.tile([128, 256], F16)
            nc.gpsimd.memset(wsrc[:], 0)

            groups = [None, None, None]

            def load_group(g, split=False):
                t = inp.tile([128, 4096], F16, tag="gin")
                base = 512 + g * 4096
                if split:
                    # two halves: first channels land sooner in the
                    # DMA-saturated startup window
                    nc.sync.dma_start(t[:, 0:2048], YALL[:, base:base + 2048])
                    nc.sync.dma_start(t[:, 2048:4096],
                                      YALL[:, base + 2048:base + 4096])
                else:
                    nc.sync.dma_start(t[:], YALL[:, base:base + 4096])
                groups[g % 3] = t



            def ch_slices(k):
                g, ci = divmod(k, G)
                if g == 0:
                    if ci == 0:
                        base, tl = 512, g0a
                    elif ci < 3:
                        base, tl = (ci - 1) * 512, g0b
                    elif ci < 5:
                        base, tl = (ci - 3) * 512, g0c
                    else:
                        base, tl = (ci - 5) * 512, g0d
                else:
                    base, tl = ci * 512, groups[g % 3]
                return [tl[:, base + q * 128: base + (q + 1) * 128]
                        for q in range(4)]

            YBs = {}
            state = {}

            def stage1(k):
                yl_s, lh_s, hl_s, hh_s = ch_slices(k)
                b = pb.tile([128, 512], F32, tag="b")
                nc.tensor.matmul(b[:, 0:256], yl_s, S0,
                                 start=True, stop=False, skip_group_check=True)
                nc.tensor.matmul(b[:, 0:256], lh_s, S1,
                                 start=False, stop=True, skip_group_check=True)
                nc.tensor.matmul(b[:, 256:512], hl_s, S0,
                                 start=True, stop=False, skip_group_check=True)
                nc.tensor.matmul(b[:, 256:512], hh_s, S1,
                                 start=False, stop=True, skip_group_check=True)
                t = ttp.tile([128, 512], F16, tag="t")
                if k % 2 == 0:
                    nc.scalar.copy(t[:], b[:])
                else:
                    nc.vector.tensor_copy(t[:], b[:])
                state[k] = t

            def stage2(k):
                g, ci = divmod(k, G)
                t = state.pop(k)
                yb = py.tile([128, 512], F32, tag="yb")
                nc.tensor.matmul(yb[:, 0:256], t[:, 0:128], S0,
                                 start=True, stop=False, skip_group_check=True)
                nc.tensor.matmul(yb[:, 0:256], t[:, 256:384], S1,
                                 start=False, stop=True, skip_group_check=True)
                nc.tensor.matmul(yb[:, 256:512], t[:, 128:256], S0,
                                 start=True, stop=False, skip_group_check=True)
                nc.tensor.matmul(yb[:, 256:512], t[:, 384:512], S1,
                                 start=False, stop=True, skip_group_check=True)
                if ci == 0:
                    YBs[g] = yp.tile([128, 4096], F16, name=f"ybo{g}",
                                     tag="yb_out")
                YB = YBs[g]
                o = ci * 512
                if k % 2 == 0:
                    nc.vector.tensor_copy(YB[:, o:o + 512], yb[:])
                else:
                    # k=63 lands here: copy2 on ACT chains directly into the
                    # ACT-issued final store (no cross-engine sem at drain)
                    nc.scalar.copy(YB[:, o:o + 512], yb[:])
                last = g == n_groups - 1
                if ci == G // 2 - 1:
                    nc.sync.dma_start(OUT[:, g, 0:2048], YB[:, 0:2048])
                elif last and ci == G - 3:
                    nc.sync.dma_start(OUT[:, g, 2048:3072], YB[:, 2048:3072])
                elif last and ci == G - 2:
                    nc.sync.dma_start(OUT[:, g, 3072:3584], YB[:, 3072:3584])
                elif ci == G - 1:
                    if last:
                        # final store issued from ACT (hw-DGE capable): runs
                        # in parallel with Sync's previous issue, skips a
                        # sem hop at the drain
                        nc.scalar.dma_start(OUT[:, g, 3584:4096],
                                            YB[:, 3584:4096])
                    else:
                        nc.sync.dma_start(OUT[:, g, 2048:4096],
                                          YB[:, 2048:4096])
                    YBs.pop(g)

            # PE warmup: ramp the clock while ch0/statics stream in
            for w in range(warmup):
                wb = pb.tile([128, 512], F32, tag="b")
                nc.tensor.matmul(wb[:, 0:256], wsrc[:, 0:128], wsrc[:, 0:256],
                                 start=True, stop=True, skip_group_check=True)

            total = n_ch
            for k in range(total + lookahead):
                if k < total:
                    g, ci = divmod(k, G)
                    if ci == 0 and g + 1 < n_groups:
                        load_group(g + 1, split=(g == 0))
                    stage1(k)
                j = k - lookahead
                if j >= 0:
                    stage2(j)

    nc.compile()
    return nc


# ---------------- host wrapper: shard, run on 8 cores, gather ----------------

_CACHED = {}


def _get_compiled():
    if "nc" not in _CACHED:
        _CACHED["nc"] = build_kernel()
        _CACHED["stats"] = build_statics()
    return _CACHED["nc"], _CACHED["stats"]


def make_in_map(Yl_b, Yhr_b, Yhi_b, ST, SIGNS=None):
    return {"YALL": pack_in(Yl_b, Yhr_b, Yhi_b, ST)}


def kernel(Yl, Yhr, Yhi):
    """Inverse DTCWT (qshift) level. Yl (8,64,128,128) f32,
    Yhr/Yhi (8,64,6,64,64) f32 -> (8,64,256,256) f32.
    Data-parallel over batch: one batch element per NeuronCore."""
    from concourse.bass_utils import run_bass_kernel_spmd

    Yl = np.asarray(Yl, dtype=np.float32)
    Yhr = np.asarray(Yhr, dtype=np.float32)
    Yhi = np.asarray(Yhi, dtype=np.float32)
    B = Yl.shape[0]
    assert B == 8, f"expected batch 8, got {B}"

    nc, (ST, SIGNS) = _get_compiled()
    in_maps = [make_in_map(Yl[b], Yhr[b], Yhi[b], ST, SIGNS)
               for b in range(B)]
    res = run_bass_kernel_spmd(nc, in_maps, core_ids=list(range(B)))
    out = np.stack([unpack_y(res.results[b]["Y"].reshape(128, 8, 8, 512))
                    for b in range(B)])
    return out
